# revision 28
# baseline (speedup 1.0000x reference)
"""Trainium2 Bass kernel for nn_MoELayerStacks (moe_routing).

Full inputs in, full output out. Data-parallel over batch across 8 cores.

Math (per batch row b):
  gate = [x[:32], x[1536:1568]] @ router_w.T + router_b           # [8]
  idx  = argmax(gate)
  l1c  = x @ l1_w[e].T + l1_b[e]   for all e                      # [8, 16]
  l1x  = clip([square(l1c[:, :15])*255/256, l1c[:, :15]], 0, 1)   # [8, 30]
  l2x  = clip(l1x @ l2_w[e].T + l2_b[e], 0, 1)                    # [8, 32]
  out  = (l2x @ out_w[e].T + out_b[e] + l1c[:, 15])[idx]          # [1]

v4 design: x rides in fp8 e3m4 (measured end-to-end rel err 0.0117 vs the
2e-2 gate; e4m3 fails at 0.023). The l1 weights stay fp16 as the stationary
operand — bass allows mixed fp16 x fp8 matmuls and the PE upconverts both to
FP22, so no weight-quantization error. The 64 router features ride in an
fp32 sidecar so the argmax sees exact gate logits (quantized routing flips
experts and fails hard).

With fp8 the kernel is PE-paced (~3.7us/block), not DMA-paced, so the
schedule priorities are: earliest possible first matmul (w1t and block-0
interleaved piece loads), a PE warmup burst so the HAM clock gate reaches
2.4 GHz before real work, act-engine-first tails (Square/Relu with
per-partition bias read PSUM on the Act engine; DVE does only the cheap
SBUF min-clips and the argmax-select), and bulk loads on the sync queue so
the Act queue (which is also the scalar-DMA HWDGE sequencer) never stalls
activations behind descriptors.

Layouts: features on partitions, batch on the free dim for l1/l2. Stacked
l1 feature index r(e,o): l1x features (k = o*8+e) at r = k for k < 64 and
r = k+8 for k >= 64; the 8 l1x_out features at r = 64+e so a lane-aligned
DVE copy can drop them into rows 64..71 of the fp32 gate-stationary tile.
Gate stationary is xre[0:73]: rows 0..63 router features, 64..71 l1x_out,
72 = ones row carrying router_b (so no 64-row memset is needed).
"""

import os
from contextlib import ExitStack

import numpy as np

import concourse.bacc as bacc
import concourse.mybir as mybir
import concourse.tile as tile

N_CORES = 8
B, L1, L2, L3, E = 16384, 3072, 15, 32, 8
RF = 32  # router feats per perspective
HALF = L1 // 2
B_SH = B // N_CORES  # 2048 rows per core
KC = L1 // 128  # 24 contraction chunks
MB = 256  # batch columns per block
NB = B_SH // MB  # 8 blocks
NSUB = MB // 128  # 2 128-col chunks per block

F32 = mybir.dt.float32
F16 = mybir.dt.float16
F8 = mybir.dt.float8e3
ALU = mybir.AluOpType
AF = mybir.ActivationFunctionType


def _stack_row(k):
    """Stacked l1 partition for l1x feature k = o*8+e (l1x_out at 64..71)."""
    return k if k < 64 else k + 8


def build_nc():
    nc = bacc.Bacc(dynamic_dma_scratch_size=2048)

    xp = nc.dram_tensor("xp", [128, NB * KC * MB], F8, kind="ExternalInput")
    xr = nc.dram_tensor("xr", [2 * RF, B_SH], F32, kind="ExternalInput")
    xo = nc.dram_tensor("xo", [1, B_SH], F32, kind="ExternalInput")
    w1t = nc.dram_tensor("w1t", [128, KC * 128], F16, kind="ExternalInput")
    cw16 = nc.dram_tensor("cw16", [128, 544], F16, kind="ExternalInput")
    cw32 = nc.dram_tensor("cw32", [128, 24], F32, kind="ExternalInput")
    y = nc.dram_tensor("y", [128, NB * NSUB], F32, kind="ExternalOutput")

    with tile.TileContext(nc) as tc, ExitStack() as ctx:
        const = ctx.enter_context(tc.tile_pool(name="const", bufs=1))
        actp = ctx.enter_context(tc.tile_pool(name="act", bufs=2))
        smallp = ctx.enter_context(tc.tile_pool(name="small", bufs=2))
        ps_1 = ctx.enter_context(tc.tile_pool(name="ps1", bufs=3, space="PSUM"))
        ps_2a = ctx.enter_context(tc.tile_pool(name="ps2a", bufs=1, space="PSUM"))
        ps_2b = ctx.enter_context(tc.tile_pool(name="ps2b", bufs=1, space="PSUM"))
        ps_sel = ctx.enter_context(tc.tile_pool(name="psel", bufs=2, space="PSUM"))
        ps_wu = ctx.enter_context(tc.tile_pool(name="pswu", bufs=1, space="PSUM"))

        w1t_sb = const.tile([128, KC, 128], F16)
        w1t_v = w1t[:, :].rearrange("p (c f) -> p c f", f=128)
        c16_sb = const.tile([128, 544], F16)
        c32_sb = const.tile([128, 24], F32)
        w2_sb = c16_sb[:, 0:512]
        w3a_sb = c16_sb[:, 512:528]
        w3b_sb = c16_sb[:, 528:544]
        wc_sb = c32_sb[:, 0:16]
        bias_sb = c32_sb[:, 16:24]
        xre = const.tile([128, B_SH], F32)  # 0..63 xr, 64..71 l1x_out, 72 ones
        yfull = const.tile([128, NB * NSUB], F32)
        wu_sb = const.tile([128, 128], F16)  # PE warmup stationary/moving
        # whole fp8 x shard stays resident: 48KB/partition. Few, large DMAs —
        # each dma_start costs ~600ns of issue time on its queue's sequencer,
        # so many small loads would pace the whole pipeline.
        xbig = const.tile([128, NB, KC, MB], F8)
        xsrc = xp[:, :].rearrange("p (b c m) -> p b c m", c=KC, m=MB)

        # --- PE warmup: trip the HAM clock gate to 8/8 before real matmuls.
        # The activity window is ~3.4us; cold matmuls at ~107ns keep the PE
        # busy from right after the framework preamble until the first real
        # burst, so block 0 runs at 2.4 GHz instead of 1.2.
        # Bridge the PE from the framework preamble (~7.5us) all the way to
        # first-data (~15.5us): ~30 cold MMs trip the HAM gate to 2.4 GHz,
        # the rest run warm at ~56ns, and the PE never re-throttles — so the
        # first real bursts run at full clock instead of 1.2 GHz.
        # ~30 cold MMs trip the HAM gate to 2.4 GHz; the rest run warm at
        # ~56ns so the warmup ends ~13.5us — within the 3.4us re-throttle
        # window of the ~16us first-data time, keeping the real bursts warm
        # without ever delaying them.
        nc.vector.memset(wu_sb[:], 0.0)
        wu_ps = ps_wu.tile([128, 128], F32, tag="wu")
        for _ in range(80):
            nc.tensor.matmul(wu_ps[:], wu_sb[:], wu_sb[:], start=True, stop=True)

        st = {}

        def emit_burst(b, c0=0, c1=KC):
            if c0 == 0:
                ps1 = ps_1.tile([128, MB], F32, tag="ps1")
                st[b] = {"ps1": ps1}
            ps1 = st[b]["ps1"]
            for c in range(c0, c1):
                nc.tensor.matmul(
                    ps1[:],
                    w1t_sb[:, c, :],
                    xbig[:, b, c, :],
                    start=(c == 0),
                    stop=(c == KC - 1),
                )

        def emit_tail(b, nsplit=1):
            ps1 = st[b]["ps1"]
            psel = ps_sel.tile([128, NSUB, 16], F32, tag="psel")
            ps2a = ps_2a.tile([128, MB], F32, tag="ps2a")
            ps2b = ps_2b.tile([128, MB], F32, tag="ps2b")
            sq = actp.tile([128, MB], F16, tag="sq")
            raw = actp.tile([128, MB], F16, tag="raw")
            l2a = actp.tile([128, MB], F16, tag="l2a")
            l2b = actp.tile([128, MB], F16, tag="l2b")
            mx = smallp.tile([128, NSUB], F32, tag="mx")
            eq = smallp.tile([128, NSUB, 8], F32, tag="eq")
            prod = smallp.tile([128, NSUB, 8], F32, tag="prod")
            w = MB // nsplit
            for s in range(nsplit):
                cs = slice(s * w, (s + 1) * w)
                m0 = b * MB + s * w
                # Act engine: PSUM-reading stage-1 with fused bias.
                # sq_t = (l1c+b1)^2 ; raw_t = relu(l1c+b1); rows 64:71 get the
                # same treatment harmlessly (their w2 rows are zero).
                nc.scalar.activation(
                    sq[:, cs], ps1[:, cs], AF.Square, bias=bias_sb[:, 1:2]
                )
                nc.scalar.activation(
                    raw[:, cs], ps1[:, cs], AF.Relu, bias=bias_sb[:, 1:2]
                )
                # l1x_out (+ l1 bias + out_b) into the fp32 gate-stationary rows
                nc.vector.tensor_scalar(
                    xre[64:72, m0 : m0 + w],
                    ps1[64:72, cs],
                    bias_sb[64:72, 4:5],
                    None,
                    op0=ALU.add,
                )
                # DVE: cheap SBUF-only clips
                nc.vector.tensor_scalar(
                    sq[:, cs], sq[:, cs], 255.0 / 256.0, 1.0, op0=ALU.mult, op1=ALU.min
                )
                nc.vector.tensor_scalar_min(raw[:, cs], raw[:, cs], 1.0)

                # l2: two expert groups (0-3, 4-7), sq+raw accumulated
                nc.tensor.matmul(
                    ps2a[:, cs], w2_sb[:, 0:128], sq[:, cs], start=True, stop=False
                )
                nc.tensor.matmul(
                    ps2a[:, cs], w2_sb[:, 128:256], raw[:, cs], start=False, stop=True
                )
                nc.tensor.matmul(
                    ps2b[:, cs], w2_sb[:, 256:384], sq[:, cs], start=True, stop=False
                )
                nc.tensor.matmul(
                    ps2b[:, cs], w2_sb[:, 384:512], raw[:, cs], start=False, stop=True
                )

                nc.scalar.activation(
                    l2a[:, cs], ps2a[:, cs], AF.Relu, bias=bias_sb[:, 2:3]
                )
                nc.scalar.activation(
                    l2b[:, cs], ps2b[:, cs], AF.Relu, bias=bias_sb[:, 3:4]
                )
                nc.vector.tensor_scalar_min(l2a[:, cs], l2a[:, cs], 1.0)
                nc.vector.tensor_scalar_min(l2b[:, cs], l2b[:, cs], 1.0)

                # batch-major gate + all_outputs: per 128-col chunk j, PSUM
                # [128, 16]: cols 0..7 gate (fp32, exact), 8..15 l1x_out + l3c
                j0, j1 = s * (NSUB // nsplit), (s + 1) * (NSUB // nsplit)
                for j in range(j0, j1):
                    c0 = b * MB + j * 128
                    nc.tensor.matmul(
                        psel[:, j, :],
                        xre[0:73, c0 : c0 + 128],
                        wc_sb[0:73, :],
                        start=True,
                        stop=False,
                    )
                    nc.tensor.matmul(
                        psel[:, j, :],
                        l2a[:, j * 128 : (j + 1) * 128],
                        w3a_sb[:],
                        start=False,
                        stop=False,
                        skip_group_check=True,
                    )
                    nc.tensor.matmul(
                        psel[:, j, :],
                        l2b[:, j * 128 : (j + 1) * 128],
                        w3b_sb[:],
                        start=False,
                        stop=True,
                        skip_group_check=True,
                    )

                # argmax-select, batch on partitions
                js = slice(j0, j1)
                nc.vector.reduce_max(
                    mx[:, js], psel[:, js, 0:8], axis=mybir.AxisListType.X
                )
                for j in range(j0, j1):
                    nc.vector.tensor_scalar(
                        eq[:, j, :],
                        psel[:, j, 0:8],
                        mx[:, j : j + 1],
                        None,
                        op0=ALU.is_ge,
                    )
                nc.vector.tensor_tensor(
                    prod[:, js], eq[:, js], psel[:, js, 8:16], op=ALU.mult
                )
                ycols = slice(b * NSUB + j0, b * NSUB + j1)
                nc.vector.reduce_sum(
                    yfull[:, ycols], prod[:, js], axis=mybir.AxisListType.X
                )
            del st[b]

        # Software pipeline, PE-paced. The sync queue carries the critical
        # lead-in (w1t front half, block-0 halves) and the early x blocks in
        # strict consumption order; the scalar queue (whose descriptor
        # stream starts ~2us later) carries the small consts and the back
        # half of x. burst(b) before tail(b-1) keeps PE priority on the l1
        # stream while the previous block's tail fills Act/DVE slack.
        # ALL bulk data rides the sync queue in exact consumption order —
        # two active queues split the 16 SDMA engines per packet and halve
        # each stream's bandwidth right when the lead-in is critical. The
        # scalar queue carries only the small consts (c32/xo tiny, then
        # xr/c16 which are not needed until the first tail).
        H = KC // 2
        nc.sync.dma_start(w1t_sb[:, 0:H, :], w1t_v[:, 0:H, :])
        nc.sync.dma_start(xbig[:, 0, 0:H, :], xsrc[:, 0, 0:H, :])
        emit_burst(0, c0=0, c1=H)
        # x0b emitted after the first sub-burst so Tile cannot merge the two
        # x0 write epochs: the first matmuls must not wait on x0b
        nc.sync.dma_start(w1t_sb[:, H:KC, :], w1t_v[:, H:KC, :])
        nc.sync.dma_start(xbig[:, 0, H:KC, :], xsrc[:, 0, H:KC, :])
        emit_burst(0, c0=H, c1=KC)
        nc.scalar.dma_start(c32_sb[:], cw32[:, :])
        # ones row for router_b: engine memsets need 32-aligned partition
        # bases, DMA does not — so the single row 72 comes in as an input
        nc.scalar.dma_start(xre[72:73, :], xo[:, :])
        nc.scalar.dma_start(xre[0 : 2 * RF, :], xr[:, :])
        nc.scalar.dma_start(c16_sb[:], cw16[:, :])
        for b in range(1, NB):
            nc.sync.dma_start(xbig[:, b, :, :], xsrc[:, b, :, :])
        for b in range(1, NB):
            emit_burst(b)
            emit_tail(b - 1)
            if b == NB - 1:
                # store the settled front of y while the last tails run
                nc.sync.dma_start(
                    y[:, 0 : (NB - 1) * NSUB], yfull[:, 0 : (NB - 1) * NSUB]
                )
        emit_tail(NB - 1, nsplit=2)
        nc.sync.dma_start(
            y[:, (NB - 1) * NSUB :], yfull[:, (NB - 1) * NSUB :]
        )

    nc.finalize()
    return nc


def prep_weights(router_w, router_b, l1_w, l1_b, l2_w, l2_b, out_w, out_b):
    """Host-side packing of the (tiny) weights into the kernel's layouts."""
    f4, f2 = np.float32, np.float16
    # stacked l1 rows: l1x k=o*8+e -> r(k); l1x_out e -> 64+e
    w1_stacked = np.zeros((128, L1), f4)
    b1col = np.zeros(128, f4)
    for o in range(L2):
        for e in range(E):
            r = _stack_row(o * 8 + e)
            w1_stacked[r] = l1_w[e, o, :]
            b1col[r] = l1_b[e, o]
    for e in range(E):
        w1_stacked[64 + e] = l1_w[e, L2, :]
        b1col[64 + e] = l1_b[e, L2]
    w1t_kf = np.ascontiguousarray(w1_stacked.T).astype(f2)  # [L1, 128]
    # swizzle to [p, c, f] so the on-chip load is contiguous per partition
    w1t = np.ascontiguousarray(
        np.transpose(w1t_kf.reshape(KC, 128, 128), (1, 0, 2))
    ).reshape(128, KC * 128)
    # l2 weights: rows r(e,o), packed [sqA | rawA | sqB | rawB]
    w2p = np.zeros((128, 512), f4)
    for e in range(E):
        base = 0 if e < 4 else 256
        c0 = (e % 4) * 32
        wt = l2_w[e].T  # [30, 32]; rows 0..14 sq features, 15..29 raw
        rows = np.array([_stack_row(o * 8 + e) for o in range(L2)])
        w2p[rows, base + c0 : base + c0 + 32] = wt[0:L2]
        w2p[rows, base + 128 + c0 : base + 128 + c0 + 32] = wt[L2 : 2 * L2]
    w2p = w2p.astype(f2)
    # l3 (batch-major): w3p[:, g*16 + 8 + e] over the 32-feature band of e
    w3p = np.zeros((128, 32), f4)
    for e in range(E):
        g = e // 4
        w3p[(e % 4) * 32 : (e % 4) * 32 + 32, g * 16 + 8 + e] = out_w[e, 0, :]
    w3p = w3p.astype(f2)
    # wcomb: rows 0..63 router_w.T -> gate cols; rows 64..71 identity -> l1x_out
    # passthrough; row 72 (ones row in xre) carries router_b
    wcp = np.zeros((128, 16), f4)
    wcp[0 : 2 * RF, 0:8] = router_w.T
    for e in range(E):
        wcp[64 + e, 8 + e] = 1.0
    wcp[72, 0:8] = router_b
    biasp = np.zeros((128, 8), f4)
    biasp[:, 1] = b1col
    biasp[:, 2] = l2_b[0:4].reshape(128)
    biasp[:, 3] = l2_b[4:8].reshape(128)
    biasp[64:72, 4] = l1_b[:, L2] + out_b[:, 0]
    cw16 = np.concatenate([w2p, w3p], axis=1)  # [128, 544] f16
    cw32 = np.concatenate([wcp, biasp], axis=1).astype(f4)  # [128, 24] f32
    return {"w1t": w1t, "cw16": cw16, "cw32": cw32}


_cache = {}
_last_results = None


def kernel(x, router_w, router_b, l1_w, l1_b, l2_w, l2_b, out_w, out_b):
    global _last_results
    x = np.asarray(x, dtype=np.float32)
    weights = prep_weights(
        np.asarray(router_w, np.float32),
        np.asarray(router_b, np.float32),
        np.asarray(l1_w, np.float32),
        np.asarray(l1_b, np.float32),
        np.asarray(l2_w, np.float32),
        np.asarray(l2_b, np.float32),
        np.asarray(out_w, np.float32),
        np.asarray(out_b, np.float32),
    )

    import ml_dtypes

    xh = x.astype(ml_dtypes.float8_e3m4)
    in_maps = []
    for core in range(N_CORES):
        shard = xh[core * B_SH : (core + 1) * B_SH]  # [2048, 3072] f8e3m4
        # xp[p, b, c, m] = shard[b*MB + m, c*128 + p]
        xp = np.ascontiguousarray(
            shard.reshape(NB, MB, KC, 128).transpose(3, 0, 2, 1)
        ).reshape(128, NB * KC * MB)
        sh32 = x[core * B_SH : (core + 1) * B_SH]
        xr = np.ascontiguousarray(
            np.concatenate([sh32[:, :RF], sh32[:, HALF : HALF + RF]], axis=1).T
        )  # [64, 2048] f32
        in_maps.append(
            {"xp": xp, "xr": xr, "xo": np.ones((1, B_SH), np.float32), **weights}
        )

    if "nc" not in _cache:
        _cache["nc"] = build_nc()
    nc = _cache["nc"]

    from concourse.bass_utils import run_bass_kernel_spmd

    trace = bool(int(os.environ.get("KERNEL_TRACE", "0")))
    try:
        res = run_bass_kernel_spmd(
            nc, in_maps, core_ids=list(range(N_CORES)), trace=trace
        )
    except Exception:
        if not trace:
            raise
        res = run_bass_kernel_spmd(
            nc, in_maps, core_ids=list(range(N_CORES)), trace=False
        )
    _last_results = res
    # y[p, g] = out row g*128 + p within the core shard
    out = np.concatenate(
        [np.ascontiguousarray(r["y"].T).reshape(B_SH, 1) for r in res.results], axis=0
    )
    return out


# revision 29
# speedup vs baseline: 1.1357x; 1.1357x over previous
"""Trainium2 Bass kernel for nn_MoELayerStacks (moe_routing).

Full inputs in, full output out. Data-parallel over batch across 8 cores.

Math (per batch row b):
  gate = [x[:32], x[1536:1568]] @ router_w.T + router_b           # [8]
  idx  = argmax(gate)
  l1c  = x @ l1_w[e].T + l1_b[e]   for all e                      # [8, 16]
  l1x  = clip([square(l1c[:, :15])*255/256, l1c[:, :15]], 0, 1)   # [8, 30]
  l2x  = clip(l1x @ l2_w[e].T + l2_b[e], 0, 1)                    # [8, 32]
  out  = (l2x @ out_w[e].T + out_b[e] + l1c[:, 15])[idx]          # [1]

v4 design: x rides in fp8 e3m4 (measured end-to-end rel err 0.0117 vs the
2e-2 gate; e4m3 fails at 0.023). The l1 weights stay fp16 as the stationary
operand — bass allows mixed fp16 x fp8 matmuls and the PE upconverts both to
FP22, so no weight-quantization error. The 64 router features ride in an
fp32 sidecar so the argmax sees exact gate logits (quantized routing flips
experts and fails hard).

With fp8 the kernel is PE-paced (~3.7us/block), not DMA-paced, so the
schedule priorities are: earliest possible first matmul (w1t and block-0
interleaved piece loads), a PE warmup burst so the HAM clock gate reaches
2.4 GHz before real work, act-engine-first tails (Square/Relu with
per-partition bias read PSUM on the Act engine; DVE does only the cheap
SBUF min-clips and the argmax-select), and bulk loads on the sync queue so
the Act queue (which is also the scalar-DMA HWDGE sequencer) never stalls
activations behind descriptors.

Layouts: features on partitions, batch on the free dim for l1/l2. Stacked
l1 feature index r(e,o): l1x features (k = o*8+e) at r = k for k < 64 and
r = k+8 for k >= 64; the 8 l1x_out features at r = 64+e so a lane-aligned
DVE copy can drop them into rows 64..71 of the fp32 gate-stationary tile.
Gate stationary is xre[0:73]: rows 0..63 router features, 64..71 l1x_out,
72 = ones row carrying router_b (so no 64-row memset is needed).
"""

import os
from contextlib import ExitStack

import numpy as np

import concourse.bacc as bacc
import concourse.mybir as mybir
import concourse.tile as tile

N_CORES = 8
B, L1, L2, L3, E = 16384, 3072, 15, 32, 8
RF = 32  # router feats per perspective
HALF = L1 // 2
B_SH = B // N_CORES  # 2048 rows per core
KC = L1 // 128  # 24 contraction chunks
MB = 256  # batch columns per block
NB = B_SH // MB  # 8 blocks
NSUB = MB // 128  # 2 128-col chunks per block

F32 = mybir.dt.float32
F16 = mybir.dt.float16
F8 = mybir.dt.float8e3
ALU = mybir.AluOpType
AF = mybir.ActivationFunctionType


def _stack_row(k):
    """Stacked l1 partition for l1x feature k = o*8+e (l1x_out at 64..71)."""
    return k if k < 64 else k + 8


def build_nc():
    nc = bacc.Bacc(dynamic_dma_scratch_size=2048)

    xp = nc.dram_tensor("xp", [128, NB * KC * MB], F8, kind="ExternalInput")
    xr = nc.dram_tensor("xr", [2 * RF, B_SH], F32, kind="ExternalInput")
    xo = nc.dram_tensor("xo", [1, B_SH], F32, kind="ExternalInput")
    w1t = nc.dram_tensor("w1t", [128, KC * 128], F16, kind="ExternalInput")
    cw16 = nc.dram_tensor("cw16", [128, 544], F16, kind="ExternalInput")
    cw32 = nc.dram_tensor("cw32", [128, 24], F32, kind="ExternalInput")
    y = nc.dram_tensor("y", [128, NB * NSUB], F32, kind="ExternalOutput")

    with tile.TileContext(nc) as tc, ExitStack() as ctx:
        const = ctx.enter_context(tc.tile_pool(name="const", bufs=1))
        actp = ctx.enter_context(tc.tile_pool(name="act", bufs=2))
        smallp = ctx.enter_context(tc.tile_pool(name="small", bufs=2))
        ps_1 = ctx.enter_context(tc.tile_pool(name="ps1", bufs=3, space="PSUM"))
        ps_2a = ctx.enter_context(tc.tile_pool(name="ps2a", bufs=1, space="PSUM"))
        ps_2b = ctx.enter_context(tc.tile_pool(name="ps2b", bufs=1, space="PSUM"))
        ps_sel = ctx.enter_context(tc.tile_pool(name="psel", bufs=2, space="PSUM"))
        ps_wu = ctx.enter_context(tc.tile_pool(name="pswu", bufs=1, space="PSUM"))

        w1t_sb = const.tile([128, KC, 128], F16)
        w1t_v = w1t[:, :].rearrange("p (c f) -> p c f", f=128)
        c16_sb = const.tile([128, 544], F16)
        c32_sb = const.tile([128, 24], F32)
        w2_sb = c16_sb[:, 0:512]
        w3a_sb = c16_sb[:, 512:528]
        w3b_sb = c16_sb[:, 528:544]
        wc_sb = c32_sb[:, 0:16]
        bias_sb = c32_sb[:, 16:24]
        xre = const.tile([128, B_SH], F32)  # 0..63 xr, 64..71 l1x_out, 72 ones
        yfull = const.tile([128, NB * NSUB], F32)
        wu_sb = const.tile([128, 128], F16)  # PE warmup stationary/moving
        # whole fp8 x shard stays resident: 48KB/partition. Few, large DMAs —
        # each dma_start costs ~600ns of issue time on its queue's sequencer,
        # so many small loads would pace the whole pipeline.
        xbig = const.tile([128, NB, KC, MB], F8)
        xsrc = xp[:, :].rearrange("p (b c m) -> p b c m", c=KC, m=MB)

        # --- PE warmup: trip the HAM clock gate to 8/8 before real matmuls.
        # The activity window is ~3.4us; cold matmuls at ~107ns keep the PE
        # busy from right after the framework preamble until the first real
        # burst, so block 0 runs at 2.4 GHz instead of 1.2.
        # Bridge the PE from the framework preamble (~7.5us) all the way to
        # first-data (~15.5us): ~30 cold MMs trip the HAM gate to 2.4 GHz,
        # the rest run warm at ~56ns, and the PE never re-throttles — so the
        # first real bursts run at full clock instead of 1.2 GHz.
        nc.vector.memset(wu_sb[:], 0.0)
        wu_ps = ps_wu.tile([128, 128], F32, tag="wu")
        for _ in range(30):
            nc.tensor.matmul(wu_ps[:], wu_sb[:], wu_sb[:], start=True, stop=True)

        st = {}

        def emit_burst(b, c0=0, c1=KC):
            if c0 == 0:
                ps1 = ps_1.tile([128, MB], F32, tag="ps1")
                st[b] = {"ps1": ps1}
            ps1 = st[b]["ps1"]
            for c in range(c0, c1):
                nc.tensor.matmul(
                    ps1[:],
                    w1t_sb[:, c, :],
                    xbig[:, b, c, :],
                    start=(c == 0),
                    stop=(c == KC - 1),
                )

        def emit_tail(b, nsplit=1):
            ps1 = st[b]["ps1"]
            psel = ps_sel.tile([128, NSUB, 16], F32, tag="psel")
            ps2a = ps_2a.tile([128, MB], F32, tag="ps2a")
            ps2b = ps_2b.tile([128, MB], F32, tag="ps2b")
            sq = actp.tile([128, MB], F16, tag="sq")
            raw = actp.tile([128, MB], F16, tag="raw")
            l2a = actp.tile([128, MB], F16, tag="l2a")
            l2b = actp.tile([128, MB], F16, tag="l2b")
            mx = smallp.tile([128, NSUB], F32, tag="mx")
            eq = smallp.tile([128, NSUB, 8], F32, tag="eq")
            prod = smallp.tile([128, NSUB, 8], F32, tag="prod")
            w = MB // nsplit
            for s in range(nsplit):
                cs = slice(s * w, (s + 1) * w)
                m0 = b * MB + s * w
                # Act engine: PSUM-reading stage-1 with fused bias.
                # sq_t = (l1c+b1)^2 ; raw_t = relu(l1c+b1); rows 64:71 get the
                # same treatment harmlessly (their w2 rows are zero).
                nc.scalar.activation(
                    sq[:, cs], ps1[:, cs], AF.Square, bias=bias_sb[:, 1:2]
                )
                nc.scalar.activation(
                    raw[:, cs], ps1[:, cs], AF.Relu, bias=bias_sb[:, 1:2]
                )
                # l1x_out (+ l1 bias + out_b) into the fp32 gate-stationary rows
                nc.vector.tensor_scalar(
                    xre[64:72, m0 : m0 + w],
                    ps1[64:72, cs],
                    bias_sb[64:72, 4:5],
                    None,
                    op0=ALU.add,
                )
                # DVE: cheap SBUF-only clips
                nc.vector.tensor_scalar(
                    sq[:, cs], sq[:, cs], 255.0 / 256.0, 1.0, op0=ALU.mult, op1=ALU.min
                )
                nc.vector.tensor_scalar_min(raw[:, cs], raw[:, cs], 1.0)

                # l2: two expert groups (0-3, 4-7), sq+raw accumulated
                nc.tensor.matmul(
                    ps2a[:, cs], w2_sb[:, 0:128], sq[:, cs], start=True, stop=False
                )
                nc.tensor.matmul(
                    ps2a[:, cs], w2_sb[:, 128:256], raw[:, cs], start=False, stop=True
                )
                nc.tensor.matmul(
                    ps2b[:, cs], w2_sb[:, 256:384], sq[:, cs], start=True, stop=False
                )
                nc.tensor.matmul(
                    ps2b[:, cs], w2_sb[:, 384:512], raw[:, cs], start=False, stop=True
                )

                nc.scalar.activation(
                    l2a[:, cs], ps2a[:, cs], AF.Relu, bias=bias_sb[:, 2:3]
                )
                nc.scalar.activation(
                    l2b[:, cs], ps2b[:, cs], AF.Relu, bias=bias_sb[:, 3:4]
                )
                nc.vector.tensor_scalar_min(l2a[:, cs], l2a[:, cs], 1.0)
                nc.vector.tensor_scalar_min(l2b[:, cs], l2b[:, cs], 1.0)

                # batch-major gate + all_outputs: per 128-col chunk j, PSUM
                # [128, 16]: cols 0..7 gate (fp32, exact), 8..15 l1x_out + l3c
                j0, j1 = s * (NSUB // nsplit), (s + 1) * (NSUB // nsplit)
                for j in range(j0, j1):
                    c0 = b * MB + j * 128
                    nc.tensor.matmul(
                        psel[:, j, :],
                        xre[0:73, c0 : c0 + 128],
                        wc_sb[0:73, :],
                        start=True,
                        stop=False,
                    )
                    nc.tensor.matmul(
                        psel[:, j, :],
                        l2a[:, j * 128 : (j + 1) * 128],
                        w3a_sb[:],
                        start=False,
                        stop=False,
                        skip_group_check=True,
                    )
                    nc.tensor.matmul(
                        psel[:, j, :],
                        l2b[:, j * 128 : (j + 1) * 128],
                        w3b_sb[:],
                        start=False,
                        stop=True,
                        skip_group_check=True,
                    )

                # argmax-select, batch on partitions
                js = slice(j0, j1)
                nc.vector.reduce_max(
                    mx[:, js], psel[:, js, 0:8], axis=mybir.AxisListType.X
                )
                for j in range(j0, j1):
                    nc.vector.tensor_scalar(
                        eq[:, j, :],
                        psel[:, j, 0:8],
                        mx[:, j : j + 1],
                        None,
                        op0=ALU.is_ge,
                    )
                nc.vector.tensor_tensor(
                    prod[:, js], eq[:, js], psel[:, js, 8:16], op=ALU.mult
                )
                ycols = slice(b * NSUB + j0, b * NSUB + j1)
                nc.vector.reduce_sum(
                    yfull[:, ycols], prod[:, js], axis=mybir.AxisListType.X
                )
            del st[b]

        # Software pipeline, PE-paced. The sync queue carries the critical
        # lead-in (w1t front half, block-0 halves) and the early x blocks in
        # strict consumption order; the scalar queue (whose descriptor
        # stream starts ~2us later) carries the small consts and the back
        # half of x. burst(b) before tail(b-1) keeps PE priority on the l1
        # stream while the previous block's tail fills Act/DVE slack.
        # ALL bulk data rides the sync queue in exact consumption order —
        # two active queues split the 16 SDMA engines per packet and halve
        # each stream's bandwidth right when the lead-in is critical. The
        # scalar queue carries only the small consts (c32/xo tiny, then
        # xr/c16 which are not needed until the first tail).
        H = KC // 2
        nc.sync.dma_start(w1t_sb[:, 0:H, :], w1t_v[:, 0:H, :])
        nc.sync.dma_start(xbig[:, 0, 0:H, :], xsrc[:, 0, 0:H, :])
        emit_burst(0, c0=0, c1=H)
        # x0b emitted after the first sub-burst so Tile cannot merge the two
        # x0 write epochs: the first matmuls must not wait on x0b
        nc.sync.dma_start(w1t_sb[:, H:KC, :], w1t_v[:, H:KC, :])
        nc.sync.dma_start(xbig[:, 0, H:KC, :], xsrc[:, 0, H:KC, :])
        emit_burst(0, c0=H, c1=KC)
        nc.scalar.dma_start(c32_sb[:], cw32[:, :])
        # ones row for router_b: engine memsets need 32-aligned partition
        # bases, DMA does not — so the single row 72 comes in as an input
        nc.scalar.dma_start(xre[72:73, :], xo[:, :])
        nc.scalar.dma_start(xre[0 : 2 * RF, :], xr[:, :])
        nc.scalar.dma_start(c16_sb[:], cw16[:, :])
        for b in range(1, NB):
            nc.sync.dma_start(xbig[:, b, :, :], xsrc[:, b, :, :])
        for b in range(1, NB):
            emit_burst(b)
            emit_tail(b - 1)
            if b == NB - 1:
                # store the settled front of y while the last tails run
                nc.sync.dma_start(
                    y[:, 0 : (NB - 1) * NSUB], yfull[:, 0 : (NB - 1) * NSUB]
                )
        emit_tail(NB - 1, nsplit=2)
        nc.sync.dma_start(
            y[:, (NB - 1) * NSUB :], yfull[:, (NB - 1) * NSUB :]
        )

    nc.finalize()
    return nc


def prep_weights(router_w, router_b, l1_w, l1_b, l2_w, l2_b, out_w, out_b):
    """Host-side packing of the (tiny) weights into the kernel's layouts."""
    f4, f2 = np.float32, np.float16
    # stacked l1 rows: l1x k=o*8+e -> r(k); l1x_out e -> 64+e
    w1_stacked = np.zeros((128, L1), f4)
    b1col = np.zeros(128, f4)
    for o in range(L2):
        for e in range(E):
            r = _stack_row(o * 8 + e)
            w1_stacked[r] = l1_w[e, o, :]
            b1col[r] = l1_b[e, o]
    for e in range(E):
        w1_stacked[64 + e] = l1_w[e, L2, :]
        b1col[64 + e] = l1_b[e, L2]
    w1t_kf = np.ascontiguousarray(w1_stacked.T).astype(f2)  # [L1, 128]
    # swizzle to [p, c, f] so the on-chip load is contiguous per partition
    w1t = np.ascontiguousarray(
        np.transpose(w1t_kf.reshape(KC, 128, 128), (1, 0, 2))
    ).reshape(128, KC * 128)
    # l2 weights: rows r(e,o), packed [sqA | rawA | sqB | rawB]
    w2p = np.zeros((128, 512), f4)
    for e in range(E):
        base = 0 if e < 4 else 256
        c0 = (e % 4) * 32
        wt = l2_w[e].T  # [30, 32]; rows 0..14 sq features, 15..29 raw
        rows = np.array([_stack_row(o * 8 + e) for o in range(L2)])
        w2p[rows, base + c0 : base + c0 + 32] = wt[0:L2]
        w2p[rows, base + 128 + c0 : base + 128 + c0 + 32] = wt[L2 : 2 * L2]
    w2p = w2p.astype(f2)
    # l3 (batch-major): w3p[:, g*16 + 8 + e] over the 32-feature band of e
    w3p = np.zeros((128, 32), f4)
    for e in range(E):
        g = e // 4
        w3p[(e % 4) * 32 : (e % 4) * 32 + 32, g * 16 + 8 + e] = out_w[e, 0, :]
    w3p = w3p.astype(f2)
    # wcomb: rows 0..63 router_w.T -> gate cols; rows 64..71 identity -> l1x_out
    # passthrough; row 72 (ones row in xre) carries router_b
    wcp = np.zeros((128, 16), f4)
    wcp[0 : 2 * RF, 0:8] = router_w.T
    for e in range(E):
        wcp[64 + e, 8 + e] = 1.0
    wcp[72, 0:8] = router_b
    biasp = np.zeros((128, 8), f4)
    biasp[:, 1] = b1col
    biasp[:, 2] = l2_b[0:4].reshape(128)
    biasp[:, 3] = l2_b[4:8].reshape(128)
    biasp[64:72, 4] = l1_b[:, L2] + out_b[:, 0]
    cw16 = np.concatenate([w2p, w3p], axis=1)  # [128, 544] f16
    cw32 = np.concatenate([wcp, biasp], axis=1).astype(f4)  # [128, 24] f32
    return {"w1t": w1t, "cw16": cw16, "cw32": cw32}


_cache = {}
_last_results = None


def kernel(x, router_w, router_b, l1_w, l1_b, l2_w, l2_b, out_w, out_b):
    global _last_results
    x = np.asarray(x, dtype=np.float32)
    weights = prep_weights(
        np.asarray(router_w, np.float32),
        np.asarray(router_b, np.float32),
        np.asarray(l1_w, np.float32),
        np.asarray(l1_b, np.float32),
        np.asarray(l2_w, np.float32),
        np.asarray(l2_b, np.float32),
        np.asarray(out_w, np.float32),
        np.asarray(out_b, np.float32),
    )

    import ml_dtypes

    xh = x.astype(ml_dtypes.float8_e3m4)
    in_maps = []
    for core in range(N_CORES):
        shard = xh[core * B_SH : (core + 1) * B_SH]  # [2048, 3072] f8e3m4
        # xp[p, b, c, m] = shard[b*MB + m, c*128 + p]
        xp = np.ascontiguousarray(
            shard.reshape(NB, MB, KC, 128).transpose(3, 0, 2, 1)
        ).reshape(128, NB * KC * MB)
        sh32 = x[core * B_SH : (core + 1) * B_SH]
        xr = np.ascontiguousarray(
            np.concatenate([sh32[:, :RF], sh32[:, HALF : HALF + RF]], axis=1).T
        )  # [64, 2048] f32
        in_maps.append(
            {"xp": xp, "xr": xr, "xo": np.ones((1, B_SH), np.float32), **weights}
        )

    if "nc" not in _cache:
        _cache["nc"] = build_nc()
    nc = _cache["nc"]

    from concourse.bass_utils import run_bass_kernel_spmd

    trace = bool(int(os.environ.get("KERNEL_TRACE", "0")))
    try:
        res = run_bass_kernel_spmd(
            nc, in_maps, core_ids=list(range(N_CORES)), trace=trace
        )
    except Exception:
        if not trace:
            raise
        res = run_bass_kernel_spmd(
            nc, in_maps, core_ids=list(range(N_CORES)), trace=False
        )
    _last_results = res
    # y[p, g] = out row g*128 + p within the core shard
    out = np.concatenate(
        [np.ascontiguousarray(r["y"].T).reshape(B_SH, 1) for r in res.results], axis=0
    )
    return out


# revision 30
# speedup vs baseline: 1.1375x; 1.0016x over previous
"""Trainium2 Bass kernel for nn_MoELayerStacks (moe_routing).

Full inputs in, full output out. Data-parallel over batch across 8 cores.

Math (per batch row b):
  gate = [x[:32], x[1536:1568]] @ router_w.T + router_b           # [8]
  idx  = argmax(gate)
  l1c  = x @ l1_w[e].T + l1_b[e]   for all e                      # [8, 16]
  l1x  = clip([square(l1c[:, :15])*255/256, l1c[:, :15]], 0, 1)   # [8, 30]
  l2x  = clip(l1x @ l2_w[e].T + l2_b[e], 0, 1)                    # [8, 32]
  out  = (l2x @ out_w[e].T + out_b[e] + l1c[:, 15])[idx]          # [1]

v4 design: x rides in fp8 e3m4 (measured end-to-end rel err 0.0117 vs the
2e-2 gate; e4m3 fails at 0.023). The l1 weights stay fp16 as the stationary
operand — bass allows mixed fp16 x fp8 matmuls and the PE upconverts both to
FP22, so no weight-quantization error. The 64 router features ride in an
fp32 sidecar so the argmax sees exact gate logits (quantized routing flips
experts and fails hard).

With fp8 the kernel is PE-paced (~3.7us/block), not DMA-paced, so the
schedule priorities are: earliest possible first matmul (w1t and block-0
interleaved piece loads), a PE warmup burst so the HAM clock gate reaches
2.4 GHz before real work, act-engine-first tails (Square/Relu with
per-partition bias read PSUM on the Act engine; DVE does only the cheap
SBUF min-clips and the argmax-select), and bulk loads on the sync queue so
the Act queue (which is also the scalar-DMA HWDGE sequencer) never stalls
activations behind descriptors.

Layouts: features on partitions, batch on the free dim for l1/l2. Stacked
l1 feature index r(e,o): l1x features (k = o*8+e) at r = k for k < 64 and
r = k+8 for k >= 64; the 8 l1x_out features at r = 64+e so a lane-aligned
DVE copy can drop them into rows 64..71 of the fp32 gate-stationary tile.
Gate stationary is xre[0:73]: rows 0..63 router features, 64..71 l1x_out,
72 = ones row carrying router_b (so no 64-row memset is needed).
"""

import os
from contextlib import ExitStack

import numpy as np

import concourse.bacc as bacc
import concourse.mybir as mybir
import concourse.tile as tile

N_CORES = 8
B, L1, L2, L3, E = 16384, 3072, 15, 32, 8
RF = 32  # router feats per perspective
HALF = L1 // 2
B_SH = B // N_CORES  # 2048 rows per core
KC = L1 // 128  # 24 contraction chunks
MB = 256  # batch columns per block
NB = B_SH // MB  # 8 blocks
NSUB = MB // 128  # 2 128-col chunks per block

F32 = mybir.dt.float32
F16 = mybir.dt.float16
F8 = mybir.dt.float8e3
ALU = mybir.AluOpType
AF = mybir.ActivationFunctionType


def _stack_row(k):
    """Stacked l1 partition for l1x feature k = o*8+e (l1x_out at 64..71)."""
    return k if k < 64 else k + 8


def build_nc():
    nc = bacc.Bacc(dynamic_dma_scratch_size=2048)

    xp = nc.dram_tensor("xp", [128, NB * KC * MB], F8, kind="ExternalInput")
    xr = nc.dram_tensor("xr", [2 * RF, B_SH], F32, kind="ExternalInput")
    xo = nc.dram_tensor("xo", [1, B_SH], F32, kind="ExternalInput")
    w1t = nc.dram_tensor("w1t", [128, KC * 128], F16, kind="ExternalInput")
    cw16 = nc.dram_tensor("cw16", [128, 544], F16, kind="ExternalInput")
    cw32 = nc.dram_tensor("cw32", [128, 24], F32, kind="ExternalInput")
    y = nc.dram_tensor("y", [128, NB * NSUB], F32, kind="ExternalOutput")

    with tile.TileContext(nc) as tc, ExitStack() as ctx:
        const = ctx.enter_context(tc.tile_pool(name="const", bufs=1))
        actp = ctx.enter_context(tc.tile_pool(name="act", bufs=2))
        smallp = ctx.enter_context(tc.tile_pool(name="small", bufs=2))
        ps_1 = ctx.enter_context(tc.tile_pool(name="ps1", bufs=3, space="PSUM"))
        ps_2a = ctx.enter_context(tc.tile_pool(name="ps2a", bufs=1, space="PSUM"))
        ps_2b = ctx.enter_context(tc.tile_pool(name="ps2b", bufs=1, space="PSUM"))
        ps_sel = ctx.enter_context(tc.tile_pool(name="psel", bufs=2, space="PSUM"))
        ps_wu = ctx.enter_context(tc.tile_pool(name="pswu", bufs=1, space="PSUM"))

        w1t_sb = const.tile([128, KC, 128], F16)
        w1t_v = w1t[:, :].rearrange("p (c f) -> p c f", f=128)
        c16_sb = const.tile([128, 544], F16)
        c32_sb = const.tile([128, 24], F32)
        w2_sb = c16_sb[:, 0:512]
        w3a_sb = c16_sb[:, 512:528]
        w3b_sb = c16_sb[:, 528:544]
        wc_sb = c32_sb[:, 0:16]
        bias_sb = c32_sb[:, 16:24]
        xre = const.tile([128, B_SH], F32)  # 0..63 xr, 64..71 l1x_out, 72 ones
        yfull = const.tile([128, NB * NSUB], F32)
        wu_sb = const.tile([128, 128], F16)  # PE warmup stationary/moving
        # whole fp8 x shard stays resident: 48KB/partition. Few, large DMAs —
        # each dma_start costs ~600ns of issue time on its queue's sequencer,
        # so many small loads would pace the whole pipeline.
        xbig = const.tile([128, NB, KC, MB], F8)
        xsrc = xp[:, :].rearrange("p (b c m) -> p b c m", c=KC, m=MB)

        # --- PE warmup: trip the HAM clock gate to 8/8 before real matmuls.
        # The activity window is ~3.4us; cold matmuls at ~107ns keep the PE
        # busy from right after the framework preamble until the first real
        # burst, so block 0 runs at 2.4 GHz instead of 1.2.
        # Bridge the PE from the framework preamble (~7.5us) all the way to
        # first-data (~15.5us): ~30 cold MMs trip the HAM gate to 2.4 GHz,
        # the rest run warm at ~56ns, and the PE never re-throttles — so the
        # first real bursts run at full clock instead of 1.2 GHz.
        nc.vector.memset(wu_sb[:], 0.0)
        wu_ps = ps_wu.tile([128, 128], F32, tag="wu")
        for _ in range(30):
            nc.tensor.matmul(wu_ps[:], wu_sb[:], wu_sb[:], start=True, stop=True)

        st = {}

        def emit_burst(b, c0=0, c1=KC):
            if c0 == 0:
                ps1 = ps_1.tile([128, MB], F32, tag="ps1")
                st[b] = {"ps1": ps1}
            ps1 = st[b]["ps1"]
            for c in range(c0, c1):
                nc.tensor.matmul(
                    ps1[:],
                    w1t_sb[:, c, :],
                    xbig[:, b, c, :],
                    start=(c == 0),
                    stop=(c == KC - 1),
                )

        def emit_tail(b, nsplit=1):
            ps1 = st[b]["ps1"]
            psel = ps_sel.tile([128, NSUB, 16], F32, tag="psel")
            ps2a = ps_2a.tile([128, MB], F32, tag="ps2a")
            ps2b = ps_2b.tile([128, MB], F32, tag="ps2b")
            sq = actp.tile([128, MB], F16, tag="sq")
            raw = actp.tile([128, MB], F16, tag="raw")
            l2a = actp.tile([128, MB], F16, tag="l2a")
            l2b = actp.tile([128, MB], F16, tag="l2b")
            mx = smallp.tile([128, NSUB], F32, tag="mx")
            eq = smallp.tile([128, NSUB, 8], F32, tag="eq")
            prod = smallp.tile([128, NSUB, 8], F32, tag="prod")
            w = MB // nsplit
            for s in range(nsplit):
                cs = slice(s * w, (s + 1) * w)
                m0 = b * MB + s * w
                # Act engine: PSUM-reading stage-1 with fused bias.
                # sq_t = (l1c+b1)^2 ; raw_t = relu(l1c+b1); rows 64:71 get the
                # same treatment harmlessly (their w2 rows are zero).
                nc.scalar.activation(
                    sq[:, cs], ps1[:, cs], AF.Square, bias=bias_sb[:, 1:2]
                )
                nc.scalar.activation(
                    raw[:, cs], ps1[:, cs], AF.Relu, bias=bias_sb[:, 1:2]
                )
                # l1x_out (+ l1 bias + out_b) into the fp32 gate-stationary rows
                nc.vector.tensor_scalar(
                    xre[64:72, m0 : m0 + w],
                    ps1[64:72, cs],
                    bias_sb[64:72, 4:5],
                    None,
                    op0=ALU.add,
                )
                # DVE: cheap SBUF-only clips
                nc.vector.tensor_scalar(
                    sq[:, cs], sq[:, cs], 255.0 / 256.0, 1.0, op0=ALU.mult, op1=ALU.min
                )
                nc.vector.tensor_scalar_min(raw[:, cs], raw[:, cs], 1.0)

                # l2: two expert groups (0-3, 4-7), sq+raw accumulated
                nc.tensor.matmul(
                    ps2a[:, cs], w2_sb[:, 0:128], sq[:, cs], start=True, stop=False
                )
                nc.tensor.matmul(
                    ps2a[:, cs], w2_sb[:, 128:256], raw[:, cs], start=False, stop=True
                )
                nc.tensor.matmul(
                    ps2b[:, cs], w2_sb[:, 256:384], sq[:, cs], start=True, stop=False
                )
                nc.tensor.matmul(
                    ps2b[:, cs], w2_sb[:, 384:512], raw[:, cs], start=False, stop=True
                )

                nc.scalar.activation(
                    l2a[:, cs], ps2a[:, cs], AF.Relu, bias=bias_sb[:, 2:3]
                )
                nc.scalar.activation(
                    l2b[:, cs], ps2b[:, cs], AF.Relu, bias=bias_sb[:, 3:4]
                )
                nc.vector.tensor_scalar_min(l2a[:, cs], l2a[:, cs], 1.0)
                nc.vector.tensor_scalar_min(l2b[:, cs], l2b[:, cs], 1.0)

                # batch-major gate + all_outputs: per 128-col chunk j, PSUM
                # [128, 16]: cols 0..7 gate (fp32, exact), 8..15 l1x_out + l3c
                j0, j1 = s * (NSUB // nsplit), (s + 1) * (NSUB // nsplit)
                for j in range(j0, j1):
                    c0 = b * MB + j * 128
                    nc.tensor.matmul(
                        psel[:, j, :],
                        xre[0:73, c0 : c0 + 128],
                        wc_sb[0:73, :],
                        start=True,
                        stop=False,
                    )
                    nc.tensor.matmul(
                        psel[:, j, :],
                        l2a[:, j * 128 : (j + 1) * 128],
                        w3a_sb[:],
                        start=False,
                        stop=False,
                        skip_group_check=True,
                    )
                    nc.tensor.matmul(
                        psel[:, j, :],
                        l2b[:, j * 128 : (j + 1) * 128],
                        w3b_sb[:],
                        start=False,
                        stop=True,
                        skip_group_check=True,
                    )

                # argmax-select, batch on partitions
                js = slice(j0, j1)
                nc.vector.reduce_max(
                    mx[:, js], psel[:, js, 0:8], axis=mybir.AxisListType.X
                )
                for j in range(j0, j1):
                    nc.vector.tensor_scalar(
                        eq[:, j, :],
                        psel[:, j, 0:8],
                        mx[:, j : j + 1],
                        None,
                        op0=ALU.is_ge,
                    )
                nc.vector.tensor_tensor(
                    prod[:, js], eq[:, js], psel[:, js, 8:16], op=ALU.mult
                )
                ycols = slice(b * NSUB + j0, b * NSUB + j1)
                nc.vector.reduce_sum(
                    yfull[:, ycols], prod[:, js], axis=mybir.AxisListType.X
                )
            del st[b]

        # Software pipeline, PE-paced. The sync queue carries the critical
        # lead-in (w1t front half, block-0 halves) and the early x blocks in
        # strict consumption order; the scalar queue (whose descriptor
        # stream starts ~2us later) carries the small consts and the back
        # half of x. burst(b) before tail(b-1) keeps PE priority on the l1
        # stream while the previous block's tail fills Act/DVE slack.
        # ALL bulk data rides the sync queue in exact consumption order —
        # two active queues split the 16 SDMA engines per packet and halve
        # each stream's bandwidth right when the lead-in is critical. The
        # scalar queue carries only the small consts (c32/xo tiny, then
        # xr/c16 which are not needed until the first tail).
        # w1t rides the (otherwise idle) GpSimd SWDGE queue — a third,
        # independent descriptor stream — so x0a is FIRST in the sync
        # queue's completion-semaphore chain and fires ~2.5us earlier.
        H = KC // 2
        nc.gpsimd.dma_start(w1t_sb[:, 0:H, :], w1t_v[:, 0:H, :])
        nc.sync.dma_start(xbig[:, 0, 0:H, :], xsrc[:, 0, 0:H, :])
        emit_burst(0, c0=0, c1=H)
        # x0b emitted after the first sub-burst so Tile cannot merge the two
        # x0 write epochs: the first matmuls must not wait on x0b
        nc.gpsimd.dma_start(w1t_sb[:, H:KC, :], w1t_v[:, H:KC, :])
        nc.sync.dma_start(xbig[:, 0, H:KC, :], xsrc[:, 0, H:KC, :])
        emit_burst(0, c0=H, c1=KC)
        nc.scalar.dma_start(c32_sb[:], cw32[:, :])
        # ones row for router_b: engine memsets need 32-aligned partition
        # bases, DMA does not — so the single row 72 comes in as an input
        nc.scalar.dma_start(xre[72:73, :], xo[:, :])
        nc.scalar.dma_start(xre[0 : 2 * RF, :], xr[:, :])
        nc.scalar.dma_start(c16_sb[:], cw16[:, :])
        for b in range(1, NB):
            nc.sync.dma_start(xbig[:, b, :, :], xsrc[:, b, :, :])
        for b in range(1, NB):
            emit_burst(b)
            emit_tail(b - 1)
            if b == NB - 1:
                # store the settled front of y while the last tails run
                nc.sync.dma_start(
                    y[:, 0 : (NB - 1) * NSUB], yfull[:, 0 : (NB - 1) * NSUB]
                )
        emit_tail(NB - 1, nsplit=2)
        nc.sync.dma_start(
            y[:, (NB - 1) * NSUB :], yfull[:, (NB - 1) * NSUB :]
        )

    nc.finalize()
    return nc


def prep_weights(router_w, router_b, l1_w, l1_b, l2_w, l2_b, out_w, out_b):
    """Host-side packing of the (tiny) weights into the kernel's layouts."""
    f4, f2 = np.float32, np.float16
    # stacked l1 rows: l1x k=o*8+e -> r(k); l1x_out e -> 64+e
    w1_stacked = np.zeros((128, L1), f4)
    b1col = np.zeros(128, f4)
    for o in range(L2):
        for e in range(E):
            r = _stack_row(o * 8 + e)
            w1_stacked[r] = l1_w[e, o, :]
            b1col[r] = l1_b[e, o]
    for e in range(E):
        w1_stacked[64 + e] = l1_w[e, L2, :]
        b1col[64 + e] = l1_b[e, L2]
    w1t_kf = np.ascontiguousarray(w1_stacked.T).astype(f2)  # [L1, 128]
    # swizzle to [p, c, f] so the on-chip load is contiguous per partition
    w1t = np.ascontiguousarray(
        np.transpose(w1t_kf.reshape(KC, 128, 128), (1, 0, 2))
    ).reshape(128, KC * 128)
    # l2 weights: rows r(e,o), packed [sqA | rawA | sqB | rawB]
    w2p = np.zeros((128, 512), f4)
    for e in range(E):
        base = 0 if e < 4 else 256
        c0 = (e % 4) * 32
        wt = l2_w[e].T  # [30, 32]; rows 0..14 sq features, 15..29 raw
        rows = np.array([_stack_row(o * 8 + e) for o in range(L2)])
        w2p[rows, base + c0 : base + c0 + 32] = wt[0:L2]
        w2p[rows, base + 128 + c0 : base + 128 + c0 + 32] = wt[L2 : 2 * L2]
    w2p = w2p.astype(f2)
    # l3 (batch-major): w3p[:, g*16 + 8 + e] over the 32-feature band of e
    w3p = np.zeros((128, 32), f4)
    for e in range(E):
        g = e // 4
        w3p[(e % 4) * 32 : (e % 4) * 32 + 32, g * 16 + 8 + e] = out_w[e, 0, :]
    w3p = w3p.astype(f2)
    # wcomb: rows 0..63 router_w.T -> gate cols; rows 64..71 identity -> l1x_out
    # passthrough; row 72 (ones row in xre) carries router_b
    wcp = np.zeros((128, 16), f4)
    wcp[0 : 2 * RF, 0:8] = router_w.T
    for e in range(E):
        wcp[64 + e, 8 + e] = 1.0
    wcp[72, 0:8] = router_b
    biasp = np.zeros((128, 8), f4)
    biasp[:, 1] = b1col
    biasp[:, 2] = l2_b[0:4].reshape(128)
    biasp[:, 3] = l2_b[4:8].reshape(128)
    biasp[64:72, 4] = l1_b[:, L2] + out_b[:, 0]
    cw16 = np.concatenate([w2p, w3p], axis=1)  # [128, 544] f16
    cw32 = np.concatenate([wcp, biasp], axis=1).astype(f4)  # [128, 24] f32
    return {"w1t": w1t, "cw16": cw16, "cw32": cw32}


_cache = {}
_last_results = None


def kernel(x, router_w, router_b, l1_w, l1_b, l2_w, l2_b, out_w, out_b):
    global _last_results
    x = np.asarray(x, dtype=np.float32)
    weights = prep_weights(
        np.asarray(router_w, np.float32),
        np.asarray(router_b, np.float32),
        np.asarray(l1_w, np.float32),
        np.asarray(l1_b, np.float32),
        np.asarray(l2_w, np.float32),
        np.asarray(l2_b, np.float32),
        np.asarray(out_w, np.float32),
        np.asarray(out_b, np.float32),
    )

    import ml_dtypes

    xh = x.astype(ml_dtypes.float8_e3m4)
    in_maps = []
    for core in range(N_CORES):
        shard = xh[core * B_SH : (core + 1) * B_SH]  # [2048, 3072] f8e3m4
        # xp[p, b, c, m] = shard[b*MB + m, c*128 + p]
        xp = np.ascontiguousarray(
            shard.reshape(NB, MB, KC, 128).transpose(3, 0, 2, 1)
        ).reshape(128, NB * KC * MB)
        sh32 = x[core * B_SH : (core + 1) * B_SH]
        xr = np.ascontiguousarray(
            np.concatenate([sh32[:, :RF], sh32[:, HALF : HALF + RF]], axis=1).T
        )  # [64, 2048] f32
        in_maps.append(
            {"xp": xp, "xr": xr, "xo": np.ones((1, B_SH), np.float32), **weights}
        )

    if "nc" not in _cache:
        _cache["nc"] = build_nc()
    nc = _cache["nc"]

    from concourse.bass_utils import run_bass_kernel_spmd

    trace = bool(int(os.environ.get("KERNEL_TRACE", "0")))
    try:
        res = run_bass_kernel_spmd(
            nc, in_maps, core_ids=list(range(N_CORES)), trace=trace
        )
    except Exception:
        if not trace:
            raise
        res = run_bass_kernel_spmd(
            nc, in_maps, core_ids=list(range(N_CORES)), trace=False
        )
    _last_results = res
    # y[p, g] = out row g*128 + p within the core shard
    out = np.concatenate(
        [np.ascontiguousarray(r["y"].T).reshape(B_SH, 1) for r in res.results], axis=0
    )
    return out
